# revision 24
# baseline (speedup 1.0000x reference)
"""Sliding-window causal self-attention (B=2, T=2048, D=1024, H=16, dk=64, W=512)
on 8 Trainium2 NeuronCores.

Sharding: core = (b, hg) for b in {0,1}, head-group hg in {0..3}.
Data parallel over batch, tensor parallel over heads: each core gets
x[b]^T, the 4-head column slices of Wq/Wk/Wv (+bq slice) and the matching
row slice of Wo, and produces a partial [T, D] output.  Host gathers with
out[b] = sum_hg partial[b,hg] + (bv @ Wo + bo).

Math notes (exact softmax identities, validated vs reference):
 - bk shifts every logit of a row by a per-row constant -> cancels in softmax.
 - bv enters the output linearly with weights summing to 1 -> folded into the
   host-side bias term bv @ Wo.
 - no max-subtraction in softmax: logits are O(1), fp32 exp is safe.

Matmuls run in float32r (single-pass fp32, 4x the fp32 rate for moving
dim >= 256; ~2e-4 matmul rel err vs 1e-7 fp32).

Device algorithm per core:
  Q^T = Wq_c^T x^T + bq_c   [256, 2048]     K^T = Wk_c^T x^T
  V   = x Wv_c              [2048, 256+ones column per head]
  per head h, per j-block J (128 keys):
    S^T[j, i] = K_h^T J-block (stationary) @ Q_h^T, i-window
                [J*128, J*128+640) clipped to T; two psum tiles (256+384)
    P^T = exp(0.125 * S^T) via ACT, two static triangular masks applied on
          the first / last 128-column blocks
  per 4-query-block group g (512 queries):
    O^T[65, 512] accumulates V_aug^T (stationary) @ P^T slices over the 8
    contributing j-blocks; row 64 = softmax denominators
    normalize: denom row -> SBUF, rank-1 ones-matmul broadcast to [64, 512],
    DVE divide, into O_hat^T
  out = O_hat^T chunks (stationary) @ Wo_c -> [2048, 1024] partial
"""

import math
from contextlib import ExitStack

import numpy as np

import concourse.bass as bass
import concourse.mybir as mybir
import concourse.tile as tile
from concourse import bacc
from concourse.bass_utils import run_bass_kernel_spmd

F32 = mybir.dt.float32
F32R = mybir.dt.float32r

T = 2048
D = 1024
NHEAD = 16
DK = 64
WINDOW = 512
HPC = 4            # heads per core
HCOLS = HPC * DK   # 256 projected columns per core
NJ = T // 128      # 16 j/query blocks
NKC = D // 128     # 8 contraction chunks over D
NG = 4             # query-block groups of 512

_NC_CACHE = {}


def _emit(tc):
    nc = tc.nc
    xT_d = nc.dram_tensor("xT", [D, T], F32R, kind="ExternalInput").ap()
    wq_d = nc.dram_tensor("wq", [D, HCOLS], F32R, kind="ExternalInput").ap()
    wk_d = nc.dram_tensor("wk", [D, HCOLS], F32R, kind="ExternalInput").ap()
    wv_d = nc.dram_tensor("wv", [D, HCOLS], F32R, kind="ExternalInput").ap()
    wo_d = nc.dram_tensor("wo", [HCOLS, D], F32R, kind="ExternalInput").ap()
    bq_d = nc.dram_tensor("bqp", [128, 2], F32, kind="ExternalInput").ap()
    mlo_d = nc.dram_tensor("mlo", [128, 128], F32R, kind="ExternalInput").ap()
    mhi_d = nc.dram_tensor("mhi", [128, 128], F32R, kind="ExternalInput").ap()
    on1_d = nc.dram_tensor("on1", [1, 64], F32R, kind="ExternalInput").ap()
    onv_d = nc.dram_tensor("onv", [128, NJ * HPC], F32R, kind="ExternalInput").ap()
    out_d = nc.dram_tensor("out", [T, D], F32, kind="ExternalOutput").ap()

    with ExitStack() as ctx:
        const_pool = ctx.enter_context(tc.tile_pool(name="const", bufs=1))
        qk_pool = ctx.enter_context(tc.tile_pool(name="qk", bufs=1))

        # ---- constants (host-provided) ----
        mask_lo = const_pool.tile([128, 128], F32R)   # keep c >= p (upper incl)
        nc.sync.dma_start(mask_lo[:], mlo_d[:, :])
        mask_hi = const_pool.tile([128, 128], F32R)   # keep c < p (strict lower)
        nc.sync.dma_start(mask_hi[:], mhi_d[:, :])
        bq_sb = const_pool.tile([128, 2], F32)
        nc.sync.dma_start(bq_sb[:], bq_d[:, :])
        ones_row = const_pool.tile([1, 64], F32R)
        nc.sync.dma_start(ones_row[:], on1_d[:, :])

        wo_sb = qk_pool.tile([128, 2, D], F32R)
        for c in range(2):
            nc.sync.dma_start(wo_sb[:, c, :], wo_d[c * 128:(c + 1) * 128, :])

        # V storage [j-part, J, head, dk+1]; col 64 of each head slot = 1.0
        v_sb = qk_pool.tile([128, NJ, HPC, DK + 1], F32R)
        nc.sync.dma_start(
            v_sb[:, :, :, DK:DK + 1].rearrange("p j h o -> p (j h o)"),
            onv_d[:, :])

        q_sb = qk_pool.tile([128, 2, T], F32R)
        k_sb = qk_pool.tile([128, 2, T], F32R)

        # ================= projections (scoped: xt + W freed after) ========
        with ExitStack() as pctx:
            xt_pool = pctx.enter_context(tc.tile_pool(name="xt", bufs=1))
            w_pool = pctx.enter_context(tc.tile_pool(name="w", bufs=1))
            ps_proj = pctx.enter_context(
                tc.tile_pool(name="ps_proj", bufs=3, space="PSUM"))

            wq_sb = w_pool.tile([128, NKC, HCOLS], F32R)
            wk_sb = w_pool.tile([128, NKC, HCOLS], F32R)
            wv_sb = w_pool.tile([128, NKC, HCOLS], F32R)
            for k in range(NKC):
                nc.sync.dma_start(wq_sb[:, k, :], wq_d[k * 128:(k + 1) * 128, :])
                nc.sync.dma_start(wk_sb[:, k, :], wk_d[k * 128:(k + 1) * 128, :])
                nc.sync.dma_start(wv_sb[:, k, :], wv_d[k * 128:(k + 1) * 128, :])

            xt_sb = xt_pool.tile([128, NKC, T], F32R)
            for n in range(4):
                for k in range(NKC):
                    nc.sync.dma_start(
                        xt_sb[:, k, n * 512:(n + 1) * 512],
                        xT_d[k * 128:(k + 1) * 128, n * 512:(n + 1) * 512],
                    )

            # Q^T, K^T (n-outer so x^T DMA overlaps compute)
            for n in range(4):
                nsl = slice(n * 512, (n + 1) * 512)
                for m in range(2):
                    qp = ps_proj.tile([128, 512], F32, tag="proj", name=f"qp{n}{m}")
                    for k in range(NKC):
                        nc.tensor.matmul(
                            qp[:], wq_sb[:, k, m * 128:(m + 1) * 128],
                            xt_sb[:, k, nsl], start=(k == 0), stop=(k == NKC - 1),
                        )
                    nc.scalar.activation(
                        q_sb[:, m, nsl], qp[:],
                        mybir.ActivationFunctionType.Identity,
                        bias=bq_sb[:, m:m + 1],
                    )
                for m in range(2):
                    kp = ps_proj.tile([128, 512], F32, tag="proj", name=f"kp{n}{m}")
                    for k in range(NKC):
                        nc.tensor.matmul(
                            kp[:], wk_sb[:, k, m * 128:(m + 1) * 128],
                            xt_sb[:, k, nsl], start=(k == 0), stop=(k == NKC - 1),
                        )
                    nc.vector.tensor_copy(k_sb[:, m, nsl], kp[:])

            # V (natural layout)
            for r in range(NJ):
                vp = ps_proj.tile([128, HPC, DK], F32, tag="proj", name=f"vp{r}")
                for k in range(NKC):
                    nc.tensor.matmul(
                        vp[:], xt_sb[:, k, r * 128:(r + 1) * 128],
                        wv_sb[:, k, :], start=(k == 0), stop=(k == NKC - 1),
                    )
                nc.vector.tensor_copy(v_sb[:, r, :, 0:DK], vp[:])

        # ================= attention =======================================
        attn_ctx = ExitStack()
        pt_pool = attn_ctx.enter_context(tc.tile_pool(name="pt", bufs=13))
        nrm_pool = attn_ctx.enter_context(tc.tile_pool(name="nrm", bufs=3))
        ps_sa = attn_ctx.enter_context(tc.tile_pool(name="ps_sa", bufs=2, space="PSUM"))
        ps_sb = attn_ctx.enter_context(tc.tile_pool(name="ps_sb", bufs=2, space="PSUM"))
        ps_pv = attn_ctx.enter_context(tc.tile_pool(name="ps_pv", bufs=2, space="PSUM"))
        ps_bc = attn_ctx.enter_context(tc.tile_pool(name="ps_bc", bufs=2, space="PSUM"))

        osb = qk_pool.tile([128, 2, T], F32R)   # normalized O^T (heads stacked)
        for h in range(HPC):
            hp = slice((h % 2) * 64, (h % 2) * 64 + 64)
            hc = h // 2
            pt_tiles = {}
            for J in range(NJ):
                width = min(640, T - J * 128)
                wA = min(256, width)
                wB = width - wA

                pt = pt_pool.tile([128, 640], F32R, tag="pt", name=f"pt_h{h}_J{J}")
                pt_tiles[J] = pt
                sa = ps_sa.tile([128, 256], F32, tag="sa", name=f"sa_h{h}_J{J}")
                nc.tensor.matmul(
                    sa[:, :wA], k_sb[hp, hc, J * 128:(J + 1) * 128],
                    q_sb[hp, hc, J * 128:J * 128 + wA],
                    start=True, stop=True,
                )
                nc.scalar.activation(
                    pt[:, 0:wA], sa[:, :wA],
                    mybir.ActivationFunctionType.Exp, scale=0.125,
                )
                if wB > 0:
                    sb = ps_sb.tile([128, 384], F32, tag="sb", name=f"sb_h{h}_J{J}")
                    nc.tensor.matmul(
                        sb[:, :wB], k_sb[hp, hc, J * 128:(J + 1) * 128],
                        q_sb[hp, hc, J * 128 + 256:J * 128 + width],
                        start=True, stop=True,
                    )
                    nc.scalar.activation(
                        pt[:, 256:256 + wB], sb[:, :wB],
                        mybir.ActivationFunctionType.Exp, scale=0.125,
                    )

                nc.vector.tensor_mul(pt[:, 0:128], pt[:, 0:128], mask_lo[:])
                if width == 640:
                    nc.vector.tensor_mul(pt[:, 512:640], pt[:, 512:640], mask_hi[:])

                if J % 4 != 3:
                    continue

                # ---- group g of 4 query blocks is fully covered: PV ----
                g = J // 4
                g0 = 512 * g
                pv = ps_pv.tile([65, 512], F32, tag="pv", name=f"pv_h{h}_g{g}")
                jps = []
                for Jp in range(max(0, 4 * g - 4), 4 * g + 4):
                    wJp = min(640, T - Jp * 128)
                    lo = max(Jp * 128, g0)
                    hi = min(Jp * 128 + wJp, g0 + 512)
                    if hi > lo:
                        jps.append((Jp, lo, hi))
                # the start=True matmul lazily zeroes the whole psum bank and
                # must fully cover it; put a full-width contribution first
                jps.sort(key=lambda t: -(t[2] - t[1]))
                assert jps[0][2] - jps[0][1] == 512
                for idx, (Jp, lo, hi) in enumerate(jps):
                    nc.tensor.matmul(
                        pv[:, lo - g0:hi - g0],
                        v_sb[:, Jp, h, :],
                        pt_tiles[Jp][:, lo - Jp * 128:hi - Jp * 128],
                        start=(idx == 0), stop=(idx == len(jps) - 1),
                    )
                for Jp in range(max(0, 4 * g - 4), 4 * g):
                    pt_tiles.pop(Jp, None)

                # ---- normalize group ----
                den = nrm_pool.tile([1, 512], F32R, tag="den", name=f"den_h{h}_g{g}")
                nc.scalar.copy(den[:], pv[64:65, :])
                bcp = ps_bc.tile([64, 512], F32, tag="bc", name=f"bc_h{h}_g{g}")
                nc.tensor.matmul(bcp[:], ones_row[:], den[:], start=True, stop=True)
                bcs = nrm_pool.tile([64, 512], F32, tag="bcs", name=f"bcs_h{h}_g{g}")
                nc.scalar.copy(bcs[:], bcp[:])
                rcp = nrm_pool.tile([64, 512], F32, tag="rcp", name=f"rcp_h{h}_g{g}")
                nc.vector.reciprocal_approx_fast(rcp[:], bcs[:])
                nc.vector.tensor_mul(
                    osb[hp, hc, g0:g0 + 512], pv[0:64, :], rcp[:],
                )

        attn_ctx.close()

        # ================= output projection ===============================
        stage_pool = ctx.enter_context(tc.tile_pool(name="stage", bufs=4))
        ps_wo = ctx.enter_context(tc.tile_pool(name="ps_wo", bufs=4, space="PSUM"))
        for qb in range(NJ):
            for nh in range(2):
                po = ps_wo.tile([128, 512], F32, tag="proj", name=f"po{qb}{nh}")
                for c in range(2):
                    nc.tensor.matmul(
                        po[:], osb[:, c, qb * 128:(qb + 1) * 128],
                        wo_sb[:, c, nh * 512:(nh + 1) * 512],
                        start=(c == 0), stop=(c == 1),
                    )
                st = stage_pool.tile([128, 512], F32, tag="stage", name=f"st{qb}{nh}")
                nc.vector.tensor_copy(st[:], po[:])
                nc.sync.dma_start(
                    out_d[qb * 128:(qb + 1) * 128, nh * 512:(nh + 1) * 512], st[:],
                )


def _build():
    if "nc" in _NC_CACHE:
        return _NC_CACHE["nc"]
    nc = bacc.Bacc("TRN2", debug=False)
    with tile.TileContext(nc) as tc:
        _emit(tc)
    nc.compile()
    _NC_CACHE["nc"] = nc
    return nc


def _shard_inputs(x, Wq, bq, Wk, Wv, Wo):
    idx = np.arange(128)
    mlo = (idx[None, :] >= idx[:, None]).astype(np.float32)  # c >= p
    mhi = (idx[None, :] < idx[:, None]).astype(np.float32)   # c < p
    on1 = np.ones((1, 64), np.float32)
    onv = np.ones((128, NJ * HPC), np.float32)
    in_maps = []
    for b in range(2):
        xT = np.ascontiguousarray(x[b].T)
        for hg in range(4):
            cols = slice(hg * HCOLS, (hg + 1) * HCOLS)
            in_maps.append({
                "xT": xT,
                "wq": np.ascontiguousarray(Wq[:, cols]),
                "wk": np.ascontiguousarray(Wk[:, cols]),
                "wv": np.ascontiguousarray(Wv[:, cols]),
                "wo": np.ascontiguousarray(Wo[cols, :]),
                "bqp": np.ascontiguousarray(bq[cols].reshape(2, 128).T),
                "mlo": mlo, "mhi": mhi, "on1": on1, "onv": onv,
            })
    return in_maps


def kernel(x, Wq, bq, Wk, bk, Wv, bv, Wo, bo, _trace=False, _tmpdir=None):
    x = np.asarray(x, dtype=np.float32)
    Wq = np.asarray(Wq, dtype=np.float32)
    Wk = np.asarray(Wk, dtype=np.float32)
    Wv = np.asarray(Wv, dtype=np.float32)
    Wo = np.asarray(Wo, dtype=np.float32)
    bq = np.asarray(bq, dtype=np.float32)
    bv = np.asarray(bv, dtype=np.float32)
    bo = np.asarray(bo, dtype=np.float32)

    nc = _build()
    in_maps = _shard_inputs(x, Wq, bq, Wk, Wv, Wo)
    res = run_bass_kernel_spmd(
        nc, in_maps, core_ids=list(range(8)), trace=_trace, tmpdir=_tmpdir,
    )
    host_bias = (bv @ Wo + bo).astype(np.float32)
    out = np.zeros((2, T, D), dtype=np.float32)
    for b in range(2):
        acc = res.results[b * 4]["out"].astype(np.float32).copy()
        for hg in range(1, 4):
            acc += res.results[b * 4 + hg]["out"]
        out[b] = acc + host_bias
    kernel._last_results = res
    return out


# revision 29
# speedup vs baseline: 1.1128x; 1.1128x over previous
"""Sliding-window causal self-attention (B=2, T=2048, D=1024, H=16, dk=64, W=512)
on 8 Trainium2 NeuronCores.

Sharding: core = (b, hg) for b in {0,1}, head-group hg in {0..3}.
Data parallel over batch, tensor parallel over heads: each core gets
x[b]^T, the 4-head column slices of Wq/Wk/Wv (+bq slice) and the matching
row slice of Wo, and produces a partial [T, D] output.  Host gathers with
out[b] = sum_hg partial[b,hg] + (bv @ Wo + bo).

Math notes (exact softmax identities, validated vs reference):
 - bk shifts every logit of a row by a per-row constant -> cancels in softmax.
 - bv enters the output linearly with weights summing to 1 -> folded into the
   host-side bias term bv @ Wo.
 - no max-subtraction in softmax: logits are O(1), fp32 exp is safe.

Matmuls run in float32r (single-pass fp32, 4x the fp32 rate for moving
dim >= 256; ~2e-4 matmul rel err vs 1e-7 fp32).

Device algorithm per core:
  Q^T = Wq_c^T x^T + bq_c   [256, 2048]     K^T = Wk_c^T x^T
  V   = x Wv_c              [2048, 256+ones column per head]
  per head h, per j-block J (128 keys):
    S^T[j, i] = K_h^T J-block (stationary) @ Q_h^T, i-window
                [J*128, J*128+640) clipped to T; two psum tiles (256+384)
    P^T = exp(0.125 * S^T) via ACT, two static triangular masks applied on
          the first / last 128-column blocks
  per 4-query-block group g (512 queries):
    O^T[65, 512] accumulates V_aug^T (stationary) @ P^T slices over the 8
    contributing j-blocks; row 64 = softmax denominators
    normalize: denom row -> SBUF, rank-1 ones-matmul broadcast to [64, 512],
    DVE divide, into O_hat^T
  out = O_hat^T chunks (stationary) @ Wo_c -> [2048, 1024] partial
"""

import math
from contextlib import ExitStack

import numpy as np

import concourse.bass as bass
import concourse.mybir as mybir
import concourse.tile as tile
from concourse import bacc
from concourse.bass_utils import run_bass_kernel_spmd

F32 = mybir.dt.float32
F32R = mybir.dt.float32r

T = 2048
D = 1024
NHEAD = 16
DK = 64
WINDOW = 512
HPC = 4            # heads per core
HCOLS = HPC * DK   # 256 projected columns per core
NJ = T // 128      # 16 j/query blocks
NKC = D // 128     # 8 contraction chunks over D
NG = 4             # query-block groups of 512

_NC_CACHE = {}


def _emit(tc):
    nc = tc.nc
    xT_d = nc.dram_tensor("xT", [D, T], F32R, kind="ExternalInput").ap()
    wq_d = nc.dram_tensor("wq", [D, HCOLS], F32R, kind="ExternalInput").ap()
    wk_d = nc.dram_tensor("wk", [D, HCOLS], F32R, kind="ExternalInput").ap()
    wv_d = nc.dram_tensor("wv", [D, HCOLS], F32R, kind="ExternalInput").ap()
    wo_d = nc.dram_tensor("wo", [HCOLS, D], F32R, kind="ExternalInput").ap()
    bq_d = nc.dram_tensor("bqp", [128, 2], F32, kind="ExternalInput").ap()
    mlo_d = nc.dram_tensor("mlo", [128, 128], F32R, kind="ExternalInput").ap()
    mhi_d = nc.dram_tensor("mhi", [128, 128], F32R, kind="ExternalInput").ap()
    onv_d = nc.dram_tensor("onv", [128, NJ * HPC], F32R, kind="ExternalInput").ap()
    out_d = nc.dram_tensor("out", [T, D], F32, kind="ExternalOutput").ap()

    with ExitStack() as ctx:
        const_pool = ctx.enter_context(tc.tile_pool(name="const", bufs=1))
        qk_pool = ctx.enter_context(tc.tile_pool(name="qk", bufs=1))

        # ---- constants (host-provided) ----
        bq_sb = const_pool.tile([128, 2], F32)
        nc.sync.dma_start(bq_sb[:], bq_d[:, :])
        mask_lo = const_pool.tile([128, 128], F32R)   # keep c >= p (upper incl)
        mask_hi = const_pool.tile([128, 128], F32R)   # keep c < p (strict lower)

        wo_sb = qk_pool.tile([128, 2, D], F32R)
        # V storage [j-part, J, head, dk+1]; col 64 of each head slot = 1.0
        v_sb = qk_pool.tile([128, NJ, HPC, DK + 1], F32R)

        q_sb = qk_pool.tile([128, 2, T], F32R)
        k_sb = qk_pool.tile([128, 2, T], F32R)

        # ================= projections (scoped: xt + W freed after) ========
        with ExitStack() as pctx:
            xt_pool = pctx.enter_context(tc.tile_pool(name="xt", bufs=1))
            w_pool = pctx.enter_context(tc.tile_pool(name="w", bufs=1))
            ps_proj = pctx.enter_context(
                tc.tile_pool(name="ps_proj", bufs=3, space="PSUM"))

            wq_sb = w_pool.tile([128, NKC, HCOLS], F32R)
            wk_sb = w_pool.tile([128, NKC, HCOLS], F32R)
            wv_sb = w_pool.tile([128, NKC, HCOLS], F32R)
            xt_sb = xt_pool.tile([128, NKC, T], F32R)

            # DMA issue order matters: first the weights + x^T columns the
            # first projection psums consume, everything else behind them.
            for k in range(NKC):
                nc.sync.dma_start(wq_sb[:, k, :], wq_d[k * 128:(k + 1) * 128, :])
                nc.sync.dma_start(wk_sb[:, k, :], wk_d[k * 128:(k + 1) * 128, :])
            for n in range(4):
                for k in range(NKC):
                    nc.sync.dma_start(
                        xt_sb[:, k, n * 512:(n + 1) * 512],
                        xT_d[k * 128:(k + 1) * 128, n * 512:(n + 1) * 512],
                    )
                if n == 0:
                    for k in range(NKC):
                        nc.sync.dma_start(wv_sb[:, k, :],
                                          wv_d[k * 128:(k + 1) * 128, :])
                elif n == 1:
                    nc.sync.dma_start(mask_lo[:], mlo_d[:, :])
                    nc.sync.dma_start(mask_hi[:], mhi_d[:, :])
                    nc.sync.dma_start(
                        v_sb[:, :, :, DK:DK + 1].rearrange("p j h o -> p (j h o)"),
                        onv_d[:, :])
                elif n == 2:
                    for c in range(2):
                        nc.sync.dma_start(wo_sb[:, c, :],
                                          wo_d[c * 128:(c + 1) * 128, :])

            # Q^T, K^T (n-outer so x^T DMA overlaps compute)
            for n in range(4):
                nsl = slice(n * 512, (n + 1) * 512)
                for m in range(2):
                    qp = ps_proj.tile([128, 512], F32, tag="proj", name=f"qp{n}{m}")
                    for k in range(NKC):
                        nc.tensor.matmul(
                            qp[:], wq_sb[:, k, m * 128:(m + 1) * 128],
                            xt_sb[:, k, nsl], start=(k == 0), stop=(k == NKC - 1),
                        )
                    nc.scalar.activation(
                        q_sb[:, m, nsl], qp[:],
                        mybir.ActivationFunctionType.Identity,
                        bias=bq_sb[:, m:m + 1],
                    )
                for m in range(2):
                    kp = ps_proj.tile([128, 512], F32, tag="proj", name=f"kp{n}{m}")
                    for k in range(NKC):
                        nc.tensor.matmul(
                            kp[:], wk_sb[:, k, m * 128:(m + 1) * 128],
                            xt_sb[:, k, nsl], start=(k == 0), stop=(k == NKC - 1),
                        )
                    nc.vector.tensor_copy(k_sb[:, m, nsl], kp[:])

            # V (natural layout)
            for r in range(NJ):
                vp = ps_proj.tile([128, HPC, DK], F32, tag="proj", name=f"vp{r}")
                for k in range(NKC):
                    nc.tensor.matmul(
                        vp[:], xt_sb[:, k, r * 128:(r + 1) * 128],
                        wv_sb[:, k, :], start=(k == 0), stop=(k == NKC - 1),
                    )
                nc.vector.tensor_copy(v_sb[:, r, :, 0:DK], vp[:])

        # ================= attention + fused output projection =============
        attn_ctx = ExitStack()
        pt_pool = attn_ctx.enter_context(tc.tile_pool(name="pt", bufs=10))
        nrm_pool = attn_ctx.enter_context(tc.tile_pool(name="nrm", bufs=3))
        stage_pool = attn_ctx.enter_context(tc.tile_pool(name="stage", bufs=4))
        ps_s = attn_ctx.enter_context(tc.tile_pool(name="ps_s", bufs=2, space="PSUM"))
        ps_pv = attn_ctx.enter_context(tc.tile_pool(name="ps_pv", bufs=2, space="PSUM"))
        ps_wo = attn_ctx.enter_context(tc.tile_pool(name="ps_wo", bufs=2, space="PSUM"))

        osb = qk_pool.tile([128, 2, T], F32R)   # normalized O^T (heads stacked)
        for h in range(HPC):
            hp = slice((h % 2) * 64, (h % 2) * 64 + 64)
            hc = h // 2
            pt_tiles = {}
            for J in range(NJ):
                width = min(640, T - J * 128)
                wA = min(512, width)
                wB = width - wA

                pt = pt_pool.tile([128, 640], F32R, tag="pt", name=f"pt_h{h}_J{J}")
                pt_tiles[J] = pt
                st_ps = ps_s.tile([128, 640], F32, tag="s", name=f"s_h{h}_J{J}")
                nc.tensor.matmul(
                    st_ps[:, 0:wA], k_sb[hp, hc, J * 128:(J + 1) * 128],
                    q_sb[hp, hc, J * 128:J * 128 + wA],
                    start=True, stop=True,
                )
                if wB > 0:
                    nc.tensor.matmul(
                        st_ps[:, 512:512 + wB], k_sb[hp, hc, J * 128:(J + 1) * 128],
                        q_sb[hp, hc, J * 128 + 512:J * 128 + width],
                        start=True, stop=True,
                    )
                nc.scalar.activation(
                    pt[:, 0:width], st_ps[:, 0:width],
                    mybir.ActivationFunctionType.Exp, scale=0.125,
                )

                nc.vector.tensor_mul(pt[:, 0:128], pt[:, 0:128], mask_lo[:])
                if width == 640:
                    nc.vector.tensor_mul(pt[:, 512:640], pt[:, 512:640], mask_hi[:])

                if J % 4 != 3:
                    continue

                # ---- group g of 4 query blocks is fully covered: PV ----
                g = J // 4
                g0 = 512 * g
                pv = ps_pv.tile([65, 512], F32, tag="pv", name=f"pv_h{h}_g{g}")
                jps = []
                for Jp in range(max(0, 4 * g - 4), 4 * g + 4):
                    wJp = min(640, T - Jp * 128)
                    lo = max(Jp * 128, g0)
                    hi = min(Jp * 128 + wJp, g0 + 512)
                    if hi > lo:
                        jps.append((Jp, lo, hi))
                # the start=True matmul lazily zeroes the whole psum bank and
                # must fully cover it; put a full-width contribution first
                jps.sort(key=lambda t: -(t[2] - t[1]))
                assert jps[0][2] - jps[0][1] == 512
                for idx, (Jp, lo, hi) in enumerate(jps):
                    nc.tensor.matmul(
                        pv[:, lo - g0:hi - g0],
                        v_sb[:, Jp, h, :],
                        pt_tiles[Jp][:, lo - Jp * 128:hi - Jp * 128],
                        start=(idx == 0), stop=(idx == len(jps) - 1),
                    )
                for Jp in range(max(0, 4 * g - 4), 4 * g):
                    pt_tiles.pop(Jp, None)

                # ---- normalize group: denom row -> bcast -> recip -> mul ----
                den = nrm_pool.tile([1, 512], F32, tag="den", name=f"den_h{h}_g{g}")
                nc.scalar.copy(den[:], pv[64:65, :])
                pb = nrm_pool.tile([64, 512], F32, tag="pb", name=f"pb_h{h}_g{g}")
                nc.gpsimd.partition_broadcast(pb[:], den[:])
                rcp = nrm_pool.tile([64, 512], F32, tag="rcp", name=f"rcp_h{h}_g{g}")
                nc.vector.reciprocal_approx_fast(rcp[:], pb[:])
                nc.vector.tensor_mul(
                    osb[hp, hc, g0:g0 + 512], pv[0:64, :], rcp[:],
                )

                # ---- last head: this group's output rows are complete ------
                if h == HPC - 1:
                    for qb in range(4 * g, 4 * g + 4):
                        for nh in range(2):
                            po = ps_wo.tile([128, 512], F32, tag="proj",
                                            name=f"po{qb}_{nh}")
                            for c in range(2):
                                nc.tensor.matmul(
                                    po[:], osb[:, c, qb * 128:(qb + 1) * 128],
                                    wo_sb[:, c, nh * 512:(nh + 1) * 512],
                                    start=(c == 0), stop=(c == 1),
                                )
                            so = stage_pool.tile([128, 512], F32, tag="stage",
                                                 name=f"so{qb}_{nh}")
                            nc.vector.tensor_copy(so[:], po[:])
                            nc.sync.dma_start(
                                out_d[qb * 128:(qb + 1) * 128,
                                      nh * 512:(nh + 1) * 512], so[:],
                            )
        attn_ctx.close()


def _build():
    if "nc" in _NC_CACHE:
        return _NC_CACHE["nc"]
    nc = bacc.Bacc("TRN2", debug=False)
    with tile.TileContext(nc) as tc:
        _emit(tc)
    nc.compile()
    _NC_CACHE["nc"] = nc
    return nc


def _shard_inputs(x, Wq, bq, Wk, Wv, Wo):
    idx = np.arange(128)
    mlo = (idx[None, :] >= idx[:, None]).astype(np.float32)  # c >= p
    mhi = (idx[None, :] < idx[:, None]).astype(np.float32)   # c < p
    onv = np.ones((128, NJ * HPC), np.float32)
    in_maps = []
    for b in range(2):
        xT = np.ascontiguousarray(x[b].T)
        for hg in range(4):
            cols = slice(hg * HCOLS, (hg + 1) * HCOLS)
            in_maps.append({
                "xT": xT,
                "wq": np.ascontiguousarray(Wq[:, cols]),
                "wk": np.ascontiguousarray(Wk[:, cols]),
                "wv": np.ascontiguousarray(Wv[:, cols]),
                "wo": np.ascontiguousarray(Wo[cols, :]),
                "bqp": np.ascontiguousarray(bq[cols].reshape(2, 128).T),
                "mlo": mlo, "mhi": mhi, "onv": onv,
            })
    return in_maps


def kernel(x, Wq, bq, Wk, bk, Wv, bv, Wo, bo, _trace=False, _tmpdir=None):
    x = np.asarray(x, dtype=np.float32)
    Wq = np.asarray(Wq, dtype=np.float32)
    Wk = np.asarray(Wk, dtype=np.float32)
    Wv = np.asarray(Wv, dtype=np.float32)
    Wo = np.asarray(Wo, dtype=np.float32)
    bq = np.asarray(bq, dtype=np.float32)
    bv = np.asarray(bv, dtype=np.float32)
    bo = np.asarray(bo, dtype=np.float32)

    nc = _build()
    in_maps = _shard_inputs(x, Wq, bq, Wk, Wv, Wo)
    res = run_bass_kernel_spmd(
        nc, in_maps, core_ids=list(range(8)), trace=_trace, tmpdir=_tmpdir,
    )
    host_bias = (bv @ Wo + bo).astype(np.float32)
    out = np.zeros((2, T, D), dtype=np.float32)
    for b in range(2):
        acc = res.results[b * 4]["out"].astype(np.float32).copy()
        for hg in range(1, 4):
            acc += res.results[b * 4 + hg]["out"]
        out[b] = acc + host_bias
    kernel._last_results = res
    return out
